# revision 1
# baseline (speedup 1.0000x reference)
"""Trainium2 Bass kernel for CausalSelfAttention2D.

Math (per batch element b):
  xn = ChannelLayerNorm(x)          # over C per spatial position
  qkv = qkv_w @ xn + qkv_b          # 1x1 conv == matmul over C
  per head h: S = (q_h^T k_h)/8 ; causal mask ; P = softmax(S)
  O_h = v_h @ P^T ; out = proj_w @ concat(O) + proj_b

Sharding: data-parallel over B (8 batch elements -> 8 cores), identical
SPMD program per core.

Host-side algebraic folds (exact):
  - ln_g folded into qkv_w columns; ln_b folded into qkv_b.
  - v-part of qkv bias folded into proj_b (softmax rows sum to 1).
  - pos_h/pos_w additive per-head scalar bias is a softmax no-op (masked
    entries are set to -FLT_MAX *after* the bias in the reference), so it
    is dropped.

On-chip layout (per core):
  x, xn:  [C=512, L=1024] as 4 tiles of [128, 1024]   (C on partitions)
  LN stats via ones-matmul column sums (partition reduction on PE).
  q, k:   [512, L] 4 tiles [128, 1152] (128 cols zero pad for i-padding)
  vT:     [L, 512] 8 tiles [128, 512] in bf16
  scores: computed transposed, S^T[j, i], per head pair (row-packed
          K=64 matmuls via tile_position); exp on ACT (scale=1/8) straight
          out of PSUM into bf16 P^T tiles; causal mask applied post-exp as
          a 0/1 triangular multiply on the diagonal 128-col block.
  AV + den: col-packed (tile_position) matmuls over j-tiles; denominator
          via ones-lhsT matmul producing a [64,i] broadcast; out = AV *
          recip(den).
  proj:   [512, 512] @ O.

Matmul dtype: float32r (TF32-like full-rate fp32 path) except AV/den which
use bf16 P^T / vT.
"""

import os
import sys
import numpy as np

import concourse.bass as bass
import concourse.mybir as mybir
import concourse.tile as tile
from concourse import bacc
from concourse.bass import ds, ts
from concourse.bass_utils import run_bass_kernel_spmd


F32 = mybir.dt.float32
F32R = mybir.dt.float32r
BF16 = mybir.dt.bfloat16
FP16 = mybir.dt.float16

B, C, H, W = 8, 512, 32, 32
L = H * W                      # 1024
HEADS = 8
DM = 512
DH = 64                        # d_head
EPS = 1e-5
NCORES = 8

LPAD = L

# scores^T chunking per j-tile t: list of (i_start, n_cols);每 chunk stays
# inside one 512-col PSUM bank of the per-head mega region.
ST_CHUNKS = {
    0: [(0, 512), (512, 512)],
    1: [(128, 512), (640, 384)],
    2: [(256, 512), (768, 256)],
    3: [(384, 512), (896, 128)],
    4: [(512, 512)],
    5: [(640, 384)],
    6: [(768, 256)],
    7: [(896, 128)],
}
ST_EXT = {t: chunks[-1][0] + chunks[-1][1] - 128 * t for t, chunks in ST_CHUNKS.items()}


def _emit(nc, tc):
    x_d = nc.dram_tensor("x", [C, L], FP16, kind="ExternalInput").ap()
    wqkvT_d = nc.dram_tensor("wqkvT", [C, 3 * DM], FP16, kind="ExternalInput").ap()
    bq_d = nc.dram_tensor("bq", [DM], F32, kind="ExternalInput").ap()
    bk_d = nc.dram_tensor("bk", [DM], F32, kind="ExternalInput").ap()
    wprojT_d = nc.dram_tensor("wprojT", [DM, C], FP16, kind="ExternalInput").ap()
    onescol_d = nc.dram_tensor("onescol", [128, 1], FP16, kind="ExternalInput").ap()
    onesrow_d = nc.dram_tensor("onesrow", [1, 128], FP16, kind="ExternalInput").ap()
    bproj_d = nc.dram_tensor("bproj", [C], F32, kind="ExternalInput").ap()
    y_d = nc.dram_tensor("y", [C, L], F32, kind="ExternalOutput").ap()

    fexp = mybir.ActivationFunctionType.Exp
    fsqrt = mybir.ActivationFunctionType.Sqrt
    fcopy = mybir.ActivationFunctionType.Copy

    with (
        tc.tile_pool(name="const", bufs=1) as cpool,
        tc.tile_pool(name="pers", bufs=1) as pers,
        tc.tile_pool(name="pT", bufs=10) as ppool,
    ):
        # ---- constants ----
        ones_col = cpool.tile([128, 1], FP16, tag="ones_col")
        nc.sync.dma_start(ones_col[:], onescol_d[:])
        ones_row = cpool.tile([1, 128], FP16, tag="ones_row")
        nc.sync.dma_start(ones_row[:], onesrow_d[:])
        ones_den = cpool.tile([128, DH], FP16, tag="ones_den")
        nc.gpsimd.memset(ones_den[:], 1.0)
        # tri[p, f] = 1.0 if f >= p else 0.0   (keep i_rel >= j_rel)
        eps128 = cpool.tile([128, 1], F32, tag="eps")
        nc.gpsimd.memset(eps128[:], EPS)
        tri = cpool.tile([128, 128], FP16, tag="tri")
        nc.gpsimd.memset(tri[:], 1.0)
        nc.gpsimd.affine_select(
            out=tri[:], in_=tri[:],
            compare_op=mybir.AluOpType.is_ge,
            fill=0.0, base=0, pattern=[[1, 128]], channel_multiplier=-1,
        )

        # ---- persistent tiles ----
        q_t = [pers.tile([128, LPAD], FP16, tag=f"q{m}", name=f"q{m}") for m in range(4)]
        k_t = [pers.tile([128, L], FP16, tag=f"k{m}", name=f"k{m}") for m in range(4)]
        # vT_ext[j, 128h:128h+64] = v^T head h; cols 128h+64:128h+128 = 1.0
        # so one [128,128] stationary computes AV (rows 0-63) and the
        # softmax denominator broadcast (rows 64-127) in a single matmul.
        vT_t = [pers.tile([128, 2 * DM], FP16, tag=f"vT{m}", name=f"vT{m}") for m in range(8)]
        o_t = [pers.tile([128, L], FP16, tag=f"o{m}", name=f"o{m}") for m in range(4)]
        wproj_t = [pers.tile([128, C], FP16, tag=f"wp{m}", name=f"wp{m}") for m in range(4)]
        bq_t = [pers.tile([128, 1], F32, tag=f"bq{m}", name=f"bq{m}") for m in range(4)]
        bk_t = [pers.tile([128, 1], F32, tag=f"bk{m}", name=f"bk{m}") for m in range(4)]
        bp_t = [pers.tile([128, 1], F32, tag=f"bp{m}", name=f"bp{m}") for m in range(4)]

        for m in range(4):
            nc.sync.dma_start(bq_t[m][:], bq_d[ds(m * 128, 128)].rearrange("(p o) -> p o", o=1))
            nc.sync.dma_start(bk_t[m][:], bk_d[ds(m * 128, 128)].rearrange("(p o) -> p o", o=1))
            nc.sync.dma_start(bp_t[m][:], bproj_d[ds(m * 128, 128)].rearrange("(p o) -> p o", o=1))
            nc.sync.dma_start(wproj_t[m][:], wprojT_d[ts(m, 128), :])

        # =========== Phase A: LayerNorm ===========
        with (
            tc.tile_pool(name="xa", bufs=1) as xpool,
            tc.tile_pool(name="sqa", bufs=2) as sqpool,
            tc.tile_pool(name="rows", bufs=1) as rpool,
            tc.tile_pool(name="xn", bufs=1) as xnpool,
            tc.tile_pool(name="wq", bufs=1) as wqpool,
            tc.tile_pool(name="psW", bufs=1, space="PSUM") as psW,
            tc.tile_pool(name="psA", bufs=3, space="PSUM") as psA,
            tc.tile_pool(name="psAb", bufs=2, space="PSUM") as psAb,
            tc.tile_pool(name="psB", bufs=2, space="PSUM") as psB,
        ):
            x_t = [xpool.tile([128, L], FP16, tag=f"x{c}", name=f"x{c}") for c in range(4)]
            for ch in range(2):
                eng = nc.sync if ch == 0 else nc.gpsimd
                for c in range(4):
                    eng.dma_start(
                        x_t[c][:, ts(ch, 512)], x_d[ts(c, 128), ts(ch, 512)]
                    )
            w_t = [wqpool.tile([128, 3 * DM], FP16, tag=f"w{c}", name=f"w{c}") for c in range(4)]
            for c in range(4):
                eng = nc.sync if c % 2 == 0 else nc.gpsimd
                eng.dma_start(w_t[c][:], wqkvT_d[ts(c, 128), :])

            # PE warmup: bursts of dep-free matmuls keep the HAM clock-gate
            # open (2.4 GHz) across the LN/DMA head where PE would idle.
            def warmup(n):
                wu = psW.tile([64, 64], F32, tag="wu", name="wu")
                for _ in range(n):
                    nc.tensor.matmul(wu[:], ones_den[:], ones_den[:],
                                     start=True, stop=True)

            warmup(64)

            # column sums of x and x^2 -> [1, 1024] stats
            sum_ps = [psA.tile([1, 512], F32, tag="stat", name=f"sum_ps{i}") for i in range(2)]
            sq_ps = [psA.tile([1, 512], F32, tag="stat", name=f"sq_ps{i}") for i in range(2)]
            sq_t = []
            for c in range(4):
                sq = sqpool.tile([128, L], FP16, tag="sq")
                for ch in range(2):
                    nc.vector.tensor_mul(
                        sq[:, ts(ch, 512)], x_t[c][:, ts(ch, 512)], x_t[c][:, ts(ch, 512)]
                    )
                sq_t.append(sq)
            for ch in range(2):
                for c in range(4):
                    nc.tensor.matmul(
                        sum_ps[ch][:], (ones_col[:]), (x_t[c][:, ts(ch, 512)]),
                        start=(c == 0), stop=(c == 3),
                    )
                for c in range(4):
                    nc.tensor.matmul(
                        sq_ps[ch][:], (ones_col[:]), (sq_t[c][:, ts(ch, 512)]),
                        start=(c == 0), stop=(c == 3),
                    )

            warmup(24)

            # stats chain in [128, 8] layout (1-partition ops are ~100x
            # slower per element; bounce through an SBUF->SBUF DMA reshape)
            stats_row = rpool.tile([1, 2 * L], F32, tag="statrow")
            s_row = rpool.tile([1, L], FP16, tag="s")
            t_row = rpool.tile([1, L], FP16, tag="t")
            for ch in range(2):
                nc.scalar.activation(stats_row[:, ts(ch, 512)], sum_ps[ch][:], fcopy, scale=1.0 / C)
                nc.scalar.activation(stats_row[:, ds(L + 512 * ch, 512)], sq_ps[ch][:], fcopy, scale=1.0 / C)
            st = rpool.tile([128, 16], F32, tag="st")       # cols 0-7 mu, 8-15 msq
            nc.sync.dma_start(st[:, ds(0, 8)], stats_row[ds(0, 1), ds(0, L)])
            nc.sync.dma_start(st[:, ds(8, 8)], stats_row[ds(0, 1), ds(L, L)])
            mu2 = rpool.tile([128, 8], F32, tag="mu2")
            nc.vector.tensor_mul(mu2[:], st[:, ds(0, 8)], st[:, ds(0, 8)])
            nc.vector.tensor_sub(mu2[:], st[:, ds(8, 8)], mu2[:])   # var
            nc.scalar.activation(mu2[:], mu2[:], fsqrt, bias=eps128[:])
            srec = rpool.tile([128, 8], F32, tag="srec")
            nc.vector.reciprocal_approx_fast(srec[:], mu2[:])
            s16 = rpool.tile([128, 16], FP16, tag="s16")    # cols 0-7 s, 8-15 t
            nc.vector.tensor_copy(s16[:, ds(0, 8)], srec[:])
            nc.vector.tensor_mul(s16[:, ds(8, 8)], st[:, ds(0, 8)], srec[:])
            nc.sync.dma_start(s_row[ds(0, 1), :], s16[:, ds(0, 8)])
            nc.sync.dma_start(t_row[ds(0, 1), :], s16[:, ds(8, 8)])

            # broadcast s,t down 128 partitions via K=1 matmul
            bs_t = rpool.tile([128, L], FP16, tag="bs")
            bt_t = rpool.tile([128, L], FP16, tag="bt")
            for ch in range(2):
                for row, dst in ((s_row, bs_t), (t_row, bt_t)):
                    ps = psAb.tile([128, 512], F32, tag="bc")
                    nc.tensor.matmul(ps[:], (ones_row[:]), (row[:, ts(ch, 512)]),
                                     start=True, stop=True)
                    nc.vector.tensor_copy(dst[:, ts(ch, 512)], ps[:])

            warmup(24)

            xn_t = []
            for c in range(4):
                xn = xnpool.tile([128, L], FP16, tag=f"xn{c}")
                nc.vector.tensor_mul(xn[:], x_t[c][:], bs_t[:])
                nc.vector.tensor_sub(xn[:], xn[:], bt_t[:])
                xn_t.append(xn)

            # =========== Phase B: qkv projections ===========
            # q[m], k[m]: [128, 1024]; vT[m8]: [128(l), 512] bf16
            for m in range(4):
                for ch in range(2):
                    for name, off, dst, bias in (
                        ("q", 0, q_t[m], bq_t[m]),
                        ("k", DM, k_t[m], bk_t[m]),
                    ):
                        ps = psB.tile([128, 512], F32, tag="mm")
                        for c in range(4):
                            nc.tensor.matmul(
                                ps[:],
                                (w_t[c][:, ds(off + m * 128, 128)]),
                                (xn_t[c][:, ts(ch, 512)]),
                                start=(c == 0), stop=(c == 3),
                            )
                        nc.vector.tensor_scalar_add(dst[:, ts(ch, 512)], ps[:], bias[:])
            for m8 in range(8):
                for h in range(8):
                    nc.gpsimd.memset(vT_t[m8][:, ds(128 * h + 64, 64)], 1.0)
                ps = psB.tile([128, 512], F32, tag="mm")
                for c in range(4):
                    nc.tensor.matmul(
                        ps[:],
                        (xn_t[c][:, ts(m8, 128)]),
                        (w_t[c][:, ds(2 * DM, DM)]),
                        start=(c == 0), stop=(c == 3),
                    )
                for h in range(8):
                    nc.vector.tensor_copy(
                        vT_t[m8][:, ds(128 * h, 64)], ps[:, ds(64 * h, 64)]
                    )

        # =========== Phase C: attention per head pair ===========
        with (
            tc.tile_pool(name="psT", bufs=3, space="PSUM") as psT,
            tc.tile_pool(name="psAV", bufs=2, space="PSUM") as psAV,
            tc.tile_pool(name="rsb", bufs=2) as rsb,
        ):
            for p in range(4):  # head pair (2p, 2p+1)
                pT_tiles = {}
                for t in range(8):
                    ext = ST_EXT[t]
                    i0 = 128 * t
                    pT = ppool.tile([128, 2048], FP16, tag="pT")
                    megas = []
                    for hh in range(2):  # head within pair
                        megas.append(psT.tile([128, 1024], F32, tag="sT",
                                              name=f"sT{p}_{t}_{hh}"))
                    # chunk-major, head-minor: consecutive matmuls hit
                    # disjoint PE row-groups and overlap in the array
                    for (ist, ncols) in ST_CHUNKS[t]:
                        for hh in range(2):
                            pb = 64 * hh
                            nc.tensor.matmul(
                                megas[hh][:, ds(ist - i0, ncols)],
                                (k_t[p][ds(pb, 64), ts(t, 128)]),
                                (q_t[p][ds(pb, 64), ds(ist, ncols)]),
                                start=True, stop=True,
                                tile_position=(pb, 0),
                            )
                    for hh in range(2):
                        nc.scalar.activation(
                            pT[:, ds(hh * 1024, ext)],
                            megas[hh][:, ds(0, ext)],
                            fexp, scale=0.125,
                        )
                        # causal mask on the diagonal 128-col block
                        # (GpSimd: idle during attention; DVE is loaded)
                        nc.gpsimd.tensor_mul(
                            pT[:, ds(hh * 1024, 128)], pT[:, ds(hh * 1024, 128)], tri[:]
                        )
                    pT_tiles[t] = pT

                # AV + denominator in one matmul per (head, chunk, j-tile):
                # stationary [vT_h | ones] -> rows 0-63 AV, rows 64-127 den
                for cch in range(2):
                    tlist = range(4) if cch == 0 else range(8)
                    avs = []
                    for hh in range(2):
                        h = 2 * p + hh
                        av = psAV.tile([128, 512], F32, tag="av",
                                       name=f"av{p}_{cch}_{hh}")
                        avs.append(av)
                        for ti, t in enumerate(tlist):
                            lo = max(cch * 512, 128 * t)
                            n = (cch + 1) * 512 - lo
                            nc.tensor.matmul(
                                av[:, ds(lo - cch * 512, n)],
                                vT_t[t][:, ds(128 * h, 128)],
                                pT_tiles[t][:, ds(hh * 1024 + lo - 128 * t, n)],
                                start=(ti == 0), stop=(ti == len(tlist) - 1),
                            )
                    for hh in range(2):
                        rec = rsb.tile([128, 512], F32, tag="rec")
                        nc.vector.reciprocal_approx_fast(rec[:], avs[hh][:, :])
                        nc.vector.tensor_mul(
                            o_t[p][ds(64 * hh, 64), ts(cch, 512)],
                            avs[hh][ds(0, 64), :], rec[ds(64, 64), :],
                        )

            # =========== Phase D: output projection ===========
            for m in range(4):
                yt = rsb.tile([128, L], F32, tag="y")
                for ch in range(2):
                    ps = psAV.tile([128, 512], F32, tag="av")
                    for c2 in range(4):
                        nc.tensor.matmul(
                            ps[:],
                            (wproj_t[c2][:, ts(m, 128)]),
                            (o_t[c2][:, ts(ch, 512)]),
                            start=(c2 == 0), stop=(c2 == 3),
                        )
                    nc.vector.tensor_scalar_add(yt[:, ts(ch, 512)], ps[:], bp_t[m][:])
                nc.sync.dma_start(y_d[ts(m, 128), :], yt[:])


_NC_CACHE = None


def build_nc():
    global _NC_CACHE
    if _NC_CACHE is None:
        nc = bacc.Bacc("TRN2", target_bir_lowering=False, debug=False)
        with tile.TileContext(nc) as tc:
            _emit(nc, tc)
        nc.compile()
        _NC_CACHE = nc
    return _NC_CACHE


def host_inputs(x, ln_g, ln_b, qkv_w, qkv_b, proj_w, proj_b, pos_h, pos_w):
    """Fold LN affine + v-bias; build per-core input maps."""
    x = np.asarray(x, np.float32)
    ln_g = np.asarray(ln_g, np.float32)
    ln_b = np.asarray(ln_b, np.float32)
    qkv_w = np.asarray(qkv_w, np.float32)
    qkv_b = np.asarray(qkv_b, np.float32)
    proj_w = np.asarray(proj_w, np.float32)
    proj_b = np.asarray(proj_b, np.float32)

    w_eff = qkv_w * ln_g[None, :]                    # [1536, 512]
    b_eff = qkv_b + qkv_w @ ln_b                     # [1536]
    wqkvT = np.ascontiguousarray(w_eff.T)            # [512, 1536]
    bq, bk, bv = b_eff[:DM], b_eff[DM:2 * DM], b_eff[2 * DM:]
    bproj = proj_b + proj_w @ bv                     # [512]
    wprojT = np.ascontiguousarray(proj_w.T)          # [512, 512]

    common = {
        "wqkvT": wqkvT.astype(np.float16), "bq": np.ascontiguousarray(bq),
        "bk": np.ascontiguousarray(bk),
        "wprojT": wprojT.astype(np.float16),
        "bproj": np.ascontiguousarray(bproj),
        "onescol": np.ones((128, 1), np.float16),
        "onesrow": np.ones((1, 128), np.float16),
    }
    in_maps = []
    for b in range(B):
        m = dict(common)
        m["x"] = np.ascontiguousarray(x[b].reshape(C, L)).astype(np.float16)
        in_maps.append(m)
    return in_maps


def kernel(x, ln_g, ln_b, qkv_w, qkv_b, proj_w, proj_b, pos_h, pos_w, **kw):
    nc = build_nc()
    in_maps = host_inputs(x, ln_g, ln_b, qkv_w, qkv_b, proj_w, proj_b, pos_h, pos_w)
    res = run_bass_kernel_spmd(nc, in_maps, core_ids=list(range(NCORES)))
    out = np.stack([res.results[b]["y"].reshape(C, H, W) for b in range(B)])
    return out.astype(np.float32)


if __name__ == "__main__":
    nc = build_nc()
    print("built + compiled ok")



# revision 7
# speedup vs baseline: 1.1140x; 1.1140x over previous
"""Trainium2 Bass kernel for CausalSelfAttention2D.

Math (per batch element b):
  xn = ChannelLayerNorm(x)          # over C per spatial position
  qkv = qkv_w @ xn + qkv_b          # 1x1 conv == matmul over C
  per head h: S = (q_h^T k_h)/8 ; causal mask ; P = softmax(S)
  O_h = v_h @ P^T ; out = proj_w @ concat(O) + proj_b

Sharding: data-parallel over B (8 batch elements -> 8 cores), identical
SPMD program per core.

Host-side algebraic folds (exact):
  - ln_g folded into qkv_w columns; ln_b folded into qkv_b.
  - v-part of qkv bias folded into proj_b (softmax rows sum to 1).
  - pos_h/pos_w additive per-head scalar bias is a softmax no-op.

v2 restructure (LN decoupling): qkv matmuls run on RAW x so the PE never
waits for the LayerNorm stats chain:
  xn[:,l] = s[l]*x[:,l] - t[l]*1   (s = 1/sigma, t = mu*s per column l)
  q = s .* (Wq x) - t (x) w1q + bq  -> Qraw = Wq x on PE immediately;
      correction = bs*Qraw - tmp, tmp[d,l] = t[l]*w1q[d] - bq[d]  (DVE)
  vT = s[l] .* (VrawT - mu (x) w1v) -> rank-1 (-mu x w1v) matmul
      accumulated straight into the Vraw PSUM group; s scaling applied by
      the (strided, per-partition-scalar) vT fill op.
Attention phase interleaves v / qk m2,m3 matmuls between score tiles so
the PE stays busy (and HAM-warm) while ACT chews through the exps.
"""

import os
import sys
import numpy as np

import concourse.bass as bass
import concourse.mybir as mybir
import concourse.tile as tile
from concourse import bacc
from concourse.bass import ds, ts
from concourse.bass_utils import run_bass_kernel_spmd


F32 = mybir.dt.float32
BF16 = mybir.dt.bfloat16
FP16 = mybir.dt.float16

B, C, H, W = 8, 512, 32, 32
L = H * W                      # 1024
HEADS = 8
DM = 512
DH = 64                        # d_head
EPS = 1e-5
NCORES = 8

# scores^T chunking per j-tile t: list of (i_start, n_cols); each chunk
# stays inside one 512-col PSUM bank of the per-head mega region.
ST_CHUNKS = {
    0: [(0, 512), (512, 512)],
    1: [(128, 512), (640, 384)],
    2: [(256, 512), (768, 256)],
    3: [(384, 512), (896, 128)],
    4: [(512, 512)],
    5: [(640, 384)],
    6: [(768, 256)],
    7: [(896, 128)],
}
ST_EXT = {t: chunks[-1][0] + chunks[-1][1] - 128 * t for t, chunks in ST_CHUNKS.items()}


def _emit(nc, tc):
    x_d = nc.dram_tensor("x", [C, L], FP16, kind="ExternalInput").ap()
    wqkvT_d = nc.dram_tensor("wqkvT", [C, 3 * DM], FP16, kind="ExternalInput").ap()
    wprojT_d = nc.dram_tensor("wprojT", [DM, C], FP16, kind="ExternalInput").ap()
    onescol_d = nc.dram_tensor("onescol", [128, 1], FP16, kind="ExternalInput").ap()
    onesrow_d = nc.dram_tensor("onesrow", [1, 128], FP16, kind="ExternalInput").ap()
    bproj_d = nc.dram_tensor("bproj", [C], F32, kind="ExternalInput").ap()
    wqb_d = nc.dram_tensor("wqb", [128, 16], F32, kind="ExternalInput").ap()
    w1v_d = nc.dram_tensor("w1v", [1, DM], FP16, kind="ExternalInput").ap()
    y_d = nc.dram_tensor("y", [C, L], F32, kind="ExternalOutput").ap()

    fexp = mybir.ActivationFunctionType.Exp
    fsqrt = mybir.ActivationFunctionType.Sqrt
    fcopy = mybir.ActivationFunctionType.Copy
    mult = mybir.AluOpType.mult
    subtract = mybir.AluOpType.subtract

    with (
        tc.tile_pool(name="const", bufs=1) as cpool,
        tc.tile_pool(name="pers", bufs=1) as pers,
        tc.tile_pool(name="pT", bufs=10) as ppool,
    ):
        # ---- constants ----
        ones_col = cpool.tile([128, 1], FP16, tag="ones_col")
        nc.sync.dma_start(ones_col[:], onescol_d[:])
        ones_row = cpool.tile([1, 128], FP16, tag="ones_row")
        nc.sync.dma_start(ones_row[:], onesrow_d[:])
        wqb = cpool.tile([128, 16], F32, tag="wqb")
        nc.scalar.dma_start(wqb[:], wqb_d[:])
        w1v = cpool.tile([1, DM], FP16, tag="w1v")
        nc.scalar.dma_start(w1v[:], w1v_d[:])
        ones_den = cpool.tile([128, DH], FP16, tag="ones_den")
        nc.gpsimd.memset(ones_den[:], 1.0)
        eps128 = cpool.tile([128, 1], F32, tag="eps")
        nc.gpsimd.memset(eps128[:], EPS)
        # tri[p, f] = 1.0 if f >= p else 0.0   (keep i_rel >= j_rel)
        tri = cpool.tile([128, 128], FP16, tag="tri")
        nc.gpsimd.memset(tri[:], 1.0)
        nc.gpsimd.affine_select(
            out=tri[:], in_=tri[:],
            compare_op=mybir.AluOpType.is_ge,
            fill=0.0, base=0, pattern=[[1, 128]], channel_multiplier=-1,
        )

        # ---- persistent tiles ----
        x_t = [pers.tile([128, L], FP16, tag=f"x{c}", name=f"x{c}") for c in range(4)]
        w_t = [pers.tile([128, 3 * DM], FP16, tag=f"w{c}", name=f"w{c}") for c in range(4)]
        q_t = [pers.tile([128, L], FP16, tag=f"q{m}", name=f"q{m}") for m in range(4)]
        k_t = [pers.tile([128, L], FP16, tag=f"k{m}", name=f"k{m}") for m in range(4)]
        # vT_t[m8][j, 128h:128h+64] = v^T head h; cols 128h+64:128h+128 = 1.0
        vT_t = [pers.tile([128, 2 * DM], FP16, tag=f"vT{m}", name=f"vT{m}") for m in range(8)]
        o_t = [pers.tile([128, L], FP16, tag=f"o{m}", name=f"o{m}") for m in range(4)]
        wproj_t = [pers.tile([128, C], FP16, tag=f"wp{m}", name=f"wp{m}") for m in range(4)]
        bp_t = [pers.tile([128, 1], F32, tag=f"bp{m}", name=f"bp{m}") for m in range(4)]
        bs_t = pers.tile([128, L], FP16, tag="bs")
        bt_t = pers.tile([128, L], FP16, tag="bt")
        tmpq_t = [pers.tile([128, L], FP16, tag=f"tq{m}", name=f"tq{m}") for m in range(4)]
        tmpk_t = [pers.tile([128, L], FP16, tag=f"tk{m}", name=f"tk{m}") for m in range(4)]
        s_row = pers.tile([1, L], FP16, tag="s_row")
        negmu_row = pers.tile([1, L], FP16, tag="negmu_row")
        srow32 = pers.tile([1, L], F32, tag="srow32")
        scol8 = pers.tile([128, 8], F32, tag="scol8")

        # ---- input DMAs (issue cost ~0.6us each; spread across queues) ----
        nc.sync.dma_start(x_t[0][:], x_d[ts(0, 128), :])
        nc.sync.dma_start(x_t[1][:], x_d[ts(1, 128), :])
        nc.gpsimd.dma_start(x_t[2][:], x_d[ts(2, 128), :])
        nc.gpsimd.dma_start(x_t[3][:], x_d[ts(3, 128), :])
        for c in range(4):
            nc.scalar.dma_start(w_t[c][:], wqkvT_d[ts(c, 128), :])
        for m in range(4):
            nc.gpsimd.dma_start(wproj_t[m][:], wprojT_d[ts(m, 128), :])
            nc.gpsimd.dma_start(
                bp_t[m][:], bproj_d[ds(m * 128, 128)].rearrange("(p o) -> p o", o=1)
            )
        # den columns of vT default to 1.0; the fill op writes the AV slots
        for m8 in range(8):
            nc.gpsimd.memset(vT_t[m8][:], 1.0)

        # =========== Phase A/B: stats + qkv m0/m1 ===========
        with (
            tc.tile_pool(name="sqa", bufs=2) as sqpool,
            tc.tile_pool(name="rows", bufs=1) as rpool,
            tc.tile_pool(name="psA", bufs=2, space="PSUM") as psA,
            tc.tile_pool(name="psAb", bufs=2, space="PSUM") as psAb,
            tc.tile_pool(name="psB", bufs=3, space="PSUM") as psB,
        ):
            # PE warmup: dep-free matmuls open the HAM clock-gate early
            def warmup(n):
                wu = psB.tile([64, 64], F32, tag="wu", name="wu", bufs=1)
                for _ in range(n):
                    nc.tensor.matmul(wu[:], ones_den[:], ones_den[:],
                                     start=True, stop=True)

            warmup(16)

            # squares on DVE, then column sums of x and x^2 via ones-matmul;
            # ACT copies interleaved per ch so psA needs only 2 bufs
            sq_t = []
            for c in range(4):
                sq = sqpool.tile([128, L], FP16, tag="sq")
                nc.vector.tensor_mul(sq[:], x_t[c][:], x_t[c][:])
                sq_t.append(sq)
            stats_row = rpool.tile([1, 2 * L], F32, tag="statrow")
            for ch in range(2):
                sum_ps = psA.tile([1, 512], F32, tag="stat", name=f"sum_ps{ch}")
                for c in range(4):
                    nc.tensor.matmul(
                        sum_ps[:], (ones_col[:]), (x_t[c][:, ts(ch, 512)]),
                        start=(c == 0), stop=(c == 3),
                    )
                sq_ps = psA.tile([1, 512], F32, tag="stat", name=f"sq_ps{ch}")
                for c in range(4):
                    nc.tensor.matmul(
                        sq_ps[:], (ones_col[:]), (sq_t[c][:, ts(ch, 512)]),
                        start=(c == 0), stop=(c == 3),
                    )
                nc.scalar.activation(stats_row[:, ts(ch, 512)], sum_ps[:], fcopy, scale=1.0 / C)
                nc.scalar.activation(stats_row[:, ds(L + 512 * ch, 512)], sq_ps[:], fcopy, scale=1.0 / C)

            warmup(40)

            # stats chain in [128, 8] layout (1-partition ops are slow;
            # bounce through SBUF->SBUF DMA reshapes)
            st = rpool.tile([128, 16], F32, tag="st")       # cols 0-7 mu, 8-15 msq
            nc.sync.dma_start(st[:, ds(0, 8)], stats_row[ds(0, 1), ds(0, L)])
            nc.sync.dma_start(st[:, ds(8, 8)], stats_row[ds(0, 1), ds(L, L)])
            mu2 = rpool.tile([128, 8], F32, tag="mu2")
            nc.vector.tensor_mul(mu2[:], st[:, ds(0, 8)], st[:, ds(0, 8)])
            nc.vector.tensor_sub(mu2[:], st[:, ds(8, 8)], mu2[:])   # var
            nc.scalar.activation(mu2[:], mu2[:], fsqrt, bias=eps128[:])
            srec = rpool.tile([128, 8], F32, tag="srec")
            nc.vector.reciprocal_approx_fast(srec[:], mu2[:])
            s16 = rpool.tile([128, 24], FP16, tag="s16")    # s | t | -mu
            nc.vector.tensor_copy(s16[:, ds(0, 8)], srec[:])
            nc.vector.tensor_mul(s16[:, ds(8, 8)], st[:, ds(0, 8)], srec[:])
            nc.vector.tensor_scalar_mul(s16[:, ds(16, 8)], st[:, ds(0, 8)], -1.0)
            t_row = rpool.tile([1, L], FP16, tag="t_row")
            nc.sync.dma_start(s_row[ds(0, 1), :], s16[:, ds(0, 8)])
            nc.sync.dma_start(t_row[ds(0, 1), :], s16[:, ds(8, 8)])
            nc.sync.dma_start(negmu_row[ds(0, 1), :], s16[:, ds(16, 8)])
            # scol8[p, m8] = s[128*m8 + p]: 8 small column DMAs from the row
            nc.sync.dma_start(srow32[ds(0, 1), :], srec[:])
            for m8 in range(8):
                eng = nc.sync if m8 % 2 == 0 else nc.gpsimd
                eng.dma_start(scol8[:, ds(m8, 1)], srow32[ds(0, 1), ds(128 * m8, 128)])

            # broadcast s,t down 128 partitions via K=1 matmul
            for ch in range(2):
                for row, dst in ((s_row, bs_t), (t_row, bt_t)):
                    ps = psAb.tile([128, 512], F32, tag="bc")
                    nc.tensor.matmul(ps[:], (ones_row[:]), (row[:, ts(ch, 512)]),
                                     start=True, stop=True)
                    nc.vector.tensor_copy(dst[:, ts(ch, 512)], ps[:])

            # tmp[d,l] = t[l]*w1[d] - b[d]  per m-tile (one dual-op TS each)
            for m in range(4):
                nc.vector.tensor_scalar(
                    tmpq_t[m][:], bt_t[:], wqb[:, ds(m, 1)], wqb[:, ds(4 + m, 1)],
                    mult, subtract,
                )
                nc.vector.tensor_scalar(
                    tmpk_t[m][:], bt_t[:], wqb[:, ds(8 + m, 1)], wqb[:, ds(12 + m, 1)],
                    mult, subtract,
                )

            def qk_group(pool, m, ch, off, dst, tmp):
                ps = pool.tile([128, 512], F32, tag="mm" if pool is psB else "av")
                for c in range(4):
                    nc.tensor.matmul(
                        ps[:],
                        (w_t[c][:, ds(off + m * 128, 128)]),
                        (x_t[c][:, ts(ch, 512)]),
                        start=(c == 0), stop=(c == 3),
                    )
                nc.vector.tensor_mul(dst[:, ts(ch, 512)], ps[:], bs_t[:, ts(ch, 512)])
                nc.vector.tensor_sub(
                    dst[:, ts(ch, 512)], dst[:, ts(ch, 512)], tmp[:, ts(ch, 512)]
                )

            for m in range(2):          # m0, m1 here; m2, m3 interleave later
                for ch in range(2):
                    qk_group(psB, m, ch, 0, q_t[m], tmpq_t[m])
                    qk_group(psB, m, ch, DM, k_t[m], tmpk_t[m])

        # =========== Phase C/D: attention + v/qk tail + proj ===========
        with (
            tc.tile_pool(name="psT2", bufs=2, space="PSUM") as psT2,
            tc.tile_pool(name="psT1", bufs=2, space="PSUM") as psT1,
            tc.tile_pool(name="psAV", bufs=2, space="PSUM") as psAV,
            tc.tile_pool(name="rsb", bufs=2) as rsb,
        ):
            pT_tiles = {}

            def S(p, t):
                ext = ST_EXT[t]
                i0 = 128 * t
                pT = ppool.tile([128, 2048], FP16, tag="pT")
                megas = []
                for hh in range(2):
                    pool, cols = (psT2, 1024) if t < 4 else (psT1, 512)
                    megas.append(pool.tile([128, cols], F32,
                                           tag="sT2" if t < 4 else "sT1",
                                           name=f"sT{p}_{t}_{hh}"))
                # chunk-major, head-minor: consecutive matmuls hit
                # disjoint PE row-groups and overlap in the array
                for (ist, ncols) in ST_CHUNKS[t]:
                    for hh in range(2):
                        pb = 64 * hh
                        nc.tensor.matmul(
                            megas[hh][:, ds(ist - i0, ncols)],
                            (k_t[p][ds(pb, 64), ts(t, 128)]),
                            (q_t[p][ds(pb, 64), ds(ist, ncols)]),
                            start=True, stop=True,
                            tile_position=(pb, 0),
                        )
                for hh in range(2):
                    nc.scalar.activation(
                        pT[:, ds(hh * 1024, ext)],
                        megas[hh][:, ds(0, ext)],
                        fexp, scale=0.125,
                    )
                    # causal mask on the diagonal 128-col block
                    nc.gpsimd.tensor_mul(
                        pT[:, ds(hh * 1024, 128)], pT[:, ds(hh * 1024, 128)], tri[:]
                    )
                pT_tiles[(p, t)] = pT

            def V(m8):
                ps = psAV.tile([128, 512], F32, tag="av", name=f"v{m8}")
                for c in range(4):
                    nc.tensor.matmul(
                        ps[:],
                        (x_t[c][:, ts(m8, 128)]),
                        (w_t[c][:, ds(2 * DM, DM)]),
                        start=(c == 0), stop=False,
                    )
                # rank-1 LN correction: += -mu[l] * w1v[d]
                nc.tensor.matmul(
                    ps[:],
                    (negmu_row[ds(0, 1), ds(128 * m8, 128)]),
                    (w1v[:]),
                    start=False, stop=True,
                )
                # strided fill of the 8 per-head AV slots, scaled by s[l]
                dst = vT_t[m8][:].rearrange("p (h c) -> p h c", c=128)[:, :, ds(0, 64)]
                src = ps[:].rearrange("p (h c) -> p h c", c=64)
                nc.vector.tensor_scalar_mul(dst, src, scol8[:, ds(m8, 1)])

            def QK(m, g):
                ch, which = g // 2, g % 2
                if which == 0:
                    qk_group(psAV, m, ch, 0, q_t[m], tmpq_t[m])
                else:
                    qk_group(psAV, m, ch, DM, k_t[m], tmpk_t[m])

            def A(p, cch):
                tlist = range(4) if cch == 0 else range(8)
                avs = []
                for hh in range(2):
                    h = 2 * p + hh
                    av = psAV.tile([128, 512], F32, tag="av",
                                   name=f"av{p}_{cch}_{hh}")
                    avs.append(av)
                    for ti, t in enumerate(tlist):
                        lo = max(cch * 512, 128 * t)
                        n = (cch + 1) * 512 - lo
                        nc.tensor.matmul(
                            av[:, ds(lo - cch * 512, n)],
                            vT_t[t][:, ds(128 * h, 128)],
                            pT_tiles[(p, t)][:, ds(hh * 1024 + lo - 128 * t, n)],
                            start=(ti == 0), stop=(ti == len(tlist) - 1),
                        )
                for hh in range(2):
                    rec = rsb.tile([128, 512], F32, tag="rec")
                    nc.vector.reciprocal_approx_fast(rec[:], avs[hh][:, :])
                    nc.vector.tensor_mul(
                        o_t[p][ds(64 * hh, 64), ts(cch, 512)],
                        avs[hh][ds(0, 64), :], rec[ds(64, 64), :],
                    )

            def P(m):
                yt = rsb.tile([128, L], F32, tag="y")
                for ch in range(2):
                    ps = psAV.tile([128, 512], F32, tag="av")
                    for c2 in range(4):
                        nc.tensor.matmul(
                            ps[:],
                            (wproj_t[c2][:, ts(m, 128)]),
                            (o_t[c2][:, ts(ch, 512)]),
                            start=(c2 == 0), stop=(c2 == 3),
                        )
                    nc.vector.tensor_scalar_add(yt[:, ts(ch, 512)], ps[:], bp_t[m][:])
                nc.sync.dma_start(y_d[ts(m, 128), :], yt[:])

            # hand-interleaved emission: PE work (V / QK m2,m3 / AV) fills
            # the gaps while ACT works through the exps
            S(0, 0); V(0); V(1)
            S(0, 1); V(2); V(3)
            S(0, 2); V(4); V(5)
            S(0, 3); V(6); V(7)
            S(0, 4); S(0, 5)
            QK(2, 0); S(0, 6); QK(2, 1); S(0, 7); QK(2, 2)
            A(0, 0)
            QK(2, 3)
            S(1, 0); S(1, 1)
            A(0, 1)
            S(1, 2); QK(3, 0); S(1, 3); QK(3, 1)
            S(1, 4); S(1, 5)
            A(1, 0)
            S(1, 6); QK(3, 2); S(1, 7); QK(3, 3)
            A(1, 1)
            S(2, 0); S(2, 1); S(2, 2); S(2, 3)
            A(2, 0)
            S(2, 4); S(2, 5); S(2, 6); S(2, 7)
            A(2, 1)
            S(3, 0); S(3, 1); S(3, 2); S(3, 3)
            A(3, 0)
            S(3, 4); S(3, 5); S(3, 6); S(3, 7)
            A(3, 1)
            for m in range(4):
                P(m)


_NC_CACHE = None


def build_nc():
    global _NC_CACHE
    if _NC_CACHE is None:
        nc = bacc.Bacc("TRN2", target_bir_lowering=False, debug=False)
        with tile.TileContext(nc) as tc:
            _emit(nc, tc)
        nc.compile()
        _NC_CACHE = nc
    return _NC_CACHE


def host_inputs(x, ln_g, ln_b, qkv_w, qkv_b, proj_w, proj_b, pos_h, pos_w):
    """Fold LN affine + v-bias; build per-core input maps."""
    x = np.asarray(x, np.float32)
    ln_g = np.asarray(ln_g, np.float32)
    ln_b = np.asarray(ln_b, np.float32)
    qkv_w = np.asarray(qkv_w, np.float32)
    qkv_b = np.asarray(qkv_b, np.float32)
    proj_w = np.asarray(proj_w, np.float32)
    proj_b = np.asarray(proj_b, np.float32)

    w_eff = qkv_w * ln_g[None, :]                    # [1536, 512]
    b_eff = qkv_b + qkv_w @ ln_b                     # [1536]
    wqkvT = np.ascontiguousarray(w_eff.T)            # [512, 1536]
    bq, bk, bv = b_eff[:DM], b_eff[DM:2 * DM], b_eff[2 * DM:]
    bproj = proj_b + proj_w @ bv                     # [512]
    wprojT = np.ascontiguousarray(proj_w.T)          # [512, 512]

    onesC = np.ones((C,), np.float32)
    w1q = w_eff[:DM] @ onesC                         # [512]
    w1k = w_eff[DM:2 * DM] @ onesC
    w1v = w_eff[2 * DM:] @ onesC
    wqb = np.zeros((128, 16), np.float32)
    for m in range(4):
        wqb[:, m] = w1q[128 * m:128 * (m + 1)]
        wqb[:, 4 + m] = bq[128 * m:128 * (m + 1)]
        wqb[:, 8 + m] = w1k[128 * m:128 * (m + 1)]
        wqb[:, 12 + m] = bk[128 * m:128 * (m + 1)]

    common = {
        "wqkvT": wqkvT.astype(np.float16),
        "wprojT": wprojT.astype(np.float16),
        "bproj": np.ascontiguousarray(bproj),
        "wqb": wqb,
        "w1v": np.ascontiguousarray(w1v[None, :]).astype(np.float16),
        "onescol": np.ones((128, 1), np.float16),
        "onesrow": np.ones((1, 128), np.float16),
    }
    in_maps = []
    for b in range(B):
        m = dict(common)
        m["x"] = np.ascontiguousarray(x[b].reshape(C, L)).astype(np.float16)
        in_maps.append(m)
    return in_maps


def kernel(x, ln_g, ln_b, qkv_w, qkv_b, proj_w, proj_b, pos_h, pos_w, **kw):
    nc = build_nc()
    in_maps = host_inputs(x, ln_g, ln_b, qkv_w, qkv_b, proj_w, proj_b, pos_h, pos_w)
    res = run_bass_kernel_spmd(nc, in_maps, core_ids=list(range(NCORES)))
    out = np.stack([res.results[b]["y"].reshape(C, H, W) for b in range(B)])
    return out.astype(np.float32)


if __name__ == "__main__":
    nc = build_nc()
    print("built + compiled ok")
